# revision 45
# baseline (speedup 1.0000x reference)
"""Multi-head attention on 8 Trainium2 NeuronCores (Bass/Tile).

Problem: x[2,2048,1024] -> qkv proj (16 heads, hd=64) -> softmax(QK^T/8)V
-> out proj.  mask is all-ones (per spec) and is ignored.

Sharding: core c in [0,8) owns heads {2c, 2c+1} for BOTH batches
(tensor-parallel QKV + attention).  An 8-core AllToAll then converts the
head-sharded attention output into a sequence-sharded full-feature
activation: core c ends up with global row chunk c (batch c//4, rows
(c%4)*512..) of all 1024 features, and computes the output projection
full-width with no all-reduce.

Device inputs (per core):
  xt      [1024, 4096] bf16   [x[0].T | x[1].T]  (d on partitions; same all cores)
  wqk     [1024,  256] bf16   [wq_heads.T * 0.125 | wk_heads.T]  (2 heads)
  bqk     [128,     2] f32    bias columns per 128-feature chunk
  wv      [1152,  128] bf16   wv_heads.T ++ 128 rows of b_v/128
  wout    [1152, 1024] bf16   w_out.T   ++ 128 rows of b_out/128
  out     [512,  1024] f32    global row chunk c, all features

qT/kT are produced feature-major ([feat, s]) so scores come out
transposed ([k, q]) and feed attn@V with no transposes anywhere.
V is produced sequence-major with a ones column per head; attn@V thus
yields [vd | sum_exp] rows and softmax normalization happens after it
on [65, 512] tiles only (reciprocal + partition-broadcast DMA + mult).

PSUM budget: scores [128,1024] x2 bufs (4 banks) + shared "ps" pool
[*,512] x4 bufs (4 banks) = 8 banks exactly.
"""

import numpy as np
import ml_dtypes
from contextlib import ExitStack

import concourse.bass as bass
import concourse.mybir as mybir
import concourse.tile as tile
from concourse import bacc
from concourse.bass_utils import run_bass_kernel_spmd

BF16 = mybir.dt.bfloat16
F32 = mybir.dt.float32
NPBF16 = ml_dtypes.bfloat16

D, H, HD, B, S = 1024, 16, 64, 2, 2048
NCORES = 8
HPC = 2              # heads per core
FPC = HPC * HD       # 128 features per core
SS = B * S           # 4096 stacked sequence (batch-major)
SC = 512             # output rows per core (post all-to-all)
NKB = S // 128       # 16 key blocks per batch
NQC = S // 512       # 4 query chunks per batch
VW = HD + 1          # v columns per head incl. ones column

Exp = mybir.ActivationFunctionType.Exp


def _build_nc(with_collective: bool = True):
    nc = bacc.Bacc("TRN2", target_bir_lowering=False, debug=False,
                   num_devices=NCORES)
    xt = nc.dram_tensor("xt", [D, SS], BF16, kind="ExternalInput").ap()
    wqk = nc.dram_tensor("wqk", [D, 2 * FPC], BF16, kind="ExternalInput").ap()
    bqk = nc.dram_tensor("bqk", [128, 2], F32, kind="ExternalInput").ap()
    wv = nc.dram_tensor("wv", [D + 128, FPC], BF16, kind="ExternalInput").ap()
    wout = nc.dram_tensor("wout", [D + 128, D], BF16, kind="ExternalInput").ap()
    onesr = nc.dram_tensor("onesr", [VW, HD], mybir.dt.float32r,
                           kind="ExternalInput").ap()
    out = nc.dram_tensor("out", [SC, D], F32, kind="ExternalOutput").ap()

    with ExitStack() as ctx:
        tc = ctx.enter_context(tile.TileContext(nc))
        persist = ctx.enter_context(tc.tile_pool(name="persist", bufs=1))
        pexp = ctx.enter_context(tc.tile_pool(name="pexp", bufs=8))
        pwork = ctx.enter_context(tc.tile_pool(name="pwork", bufs=3))
        pbig = ctx.enter_context(tc.tile_pool(name="pbig", bufs=2, space="PSUM"))
        pso = ctx.enter_context(tc.tile_pool(name="pso", bufs=2, space="PSUM"))
        pps = ctx.enter_context(tc.tile_pool(name="pps", bufs=2, space="PSUM"))
        dram = ctx.enter_context(tc.tile_pool(name="dram", bufs=2, space="DRAM"))

        # ---------------- persistent SBUF ----------------
        # xt_sb[b][cq]: [128, 8*512] — d-chunk i at cols i*512, loaded in
        # ONE strided DMA per (batch, col-chunk) to minimize issue count
        xt_sb = [[persist.tile([128, 8 * 512], BF16, tag=f"xt{b}_{cq}",
                               name=f"xt{b}_{cq}") for cq in range(4)]
                 for b in range(B)]
        ones_full = persist.tile([128, S], BF16, tag="ones", name="ones")
        nc.vector.memset(ones_full, 1.0)

        # wqk_sb: [128, 8*256] — d-chunk kk at cols kk*256, one DMA
        wqk_sb = persist.tile([128, 8 * 2 * FPC], BF16, tag="wqk",
                              name="wqk_sb")
        bqk_sb = persist.tile([128, 2], F32, tag="bqk", name="bqk")
        wv_sb = persist.tile([128, 9 * FPC], BF16, tag="wv", name="wv_sb")
        wout_big = persist.tile([128, 9 * D], BF16, tag="wout",
                                name="wout_big")
        wout_sb = [wout_big[:, i * D:(i + 1) * D] for i in range(9)]
        # qk_sb[b][m][qn]: m 0 = qT, 1 = kT (2 heads on partition halves)
        qk_sb = [[[persist.tile([128, 512], BF16, tag=f"qk{b}_{m}_{qn}",
                                name=f"qk{b}_{m}_{qn}") for qn in range(NQC)]
                  for m in range(2)]
                 for b in range(B)]
        v_sb = [persist.tile([128, HPC * VW], BF16, tag=f"v{i}", name=f"v{i}")
                for i in range(2 * NKB)]
        # att_sb[b][h]: [64, 2048]
        att_sb = [[persist.tile([64, S], BF16, tag=f"att{b}_{h}",
                                name=f"att{b}_{h}") for h in range(HPC)]
                  for b in range(B)]
        # ones row living on partition 64 (K operand of the broadcast matmul)
        ones32 = persist.tile([VW, HD], mybir.dt.float32r, tag="ones32",
                              name="ones32")
        nc.sync.dma_start(out=ones32, in_=onesr[:, :])
        # ao_sb[0..3]: even-head (phase 1) K-chunks, [4..7]: odd-head
        ao_sb = [persist.tile([128, SC], BF16, tag=f"ao{i}", name=f"ao{i}")
                 for i in range(8)]
        part_sb = [persist.tile([128, 512], BF16, tag=f"part{g}",
                                name=f"part{g}") for g in range(8)]
        for vt in v_sb:
            nc.vector.memset(
                vt.rearrange("p (h w) -> p h w", h=HPC)[:, :, HD:VW], 1.0)

        # ---------------- loads (ordered by first use) ----------------
        def xt_chunk_ap(b, cq):
            # (p, i, s) -> xt[i*128 + p, b*S + cq*512 + s]
            return bass.AP(tensor=xt.tensor, offset=b * S + cq * 512,
                           ap=[[SS, 128], [128 * SS, 8], [1, 512]])

        nc.sync.dma_start(out=bqk_sb, in_=bqk[:, :])
        wqk_src = bass.AP(tensor=wqk.tensor, offset=0,
                          ap=[[2 * FPC, 128], [128 * 2 * FPC, 8], [1, 2 * FPC]])
        nc.sync.dma_start(
            out=wqk_sb.rearrange("p (i f) -> p i f", i=8), in_=wqk_src)
        for cq in range(2):
            nc.sync.dma_start(
                out=xt_sb[0][cq].rearrange("p (i s) -> p i s", i=8),
                in_=xt_chunk_ap(0, cq))
        wv_src = bass.AP(tensor=wv.tensor, offset=0,
                         ap=[[FPC, 128], [128 * FPC, 9], [1, FPC]])
        nc.sync.dma_start(
            out=wv_sb.rearrange("p (i f) -> p i f", i=9), in_=wv_src)
        for cq in range(2, 4):
            nc.sync.dma_start(
                out=xt_sb[0][cq].rearrange("p (i s) -> p i s", i=8),
                in_=xt_chunk_ap(0, cq))
        for cq in range(4):
            nc.sync.dma_start(
                out=xt_sb[1][cq].rearrange("p (i s) -> p i s", i=8),
                in_=xt_chunk_ap(1, cq))
        wout_src = bass.AP(tensor=wout.tensor, offset=0,
                           ap=[[D, 128], [128 * D, 9], [1, D]])
        nc.sync.dma_start(
            out=wout_big.rearrange("p (i f) -> p i f", i=9), in_=wout_src)

        a2a_in = [dram.tile([8, HD, SC], BF16, tag=f"a2a_in{h}",
                            name=f"a2a_in{h}", bufs=1) for h in range(HPC)]
        a2a_out = [dram.tile([8, HD, SC], BF16, tag=f"a2a_out{h}",
                             name=f"a2a_out{h}", bufs=1) for h in range(HPC)]

        def emit_a2a(h):
            if with_collective:
                nc.gpsimd.collective_compute(
                    "AllToAll", mybir.AluOpType.bypass,
                    replica_groups=[list(range(8))],
                    ins=[a2a_in[h][:, :, :].opt()],
                    outs=[a2a_out[h][:, :, :].opt()])

        # ------------- projections + attention, interleaved -------------
        def emit_qk(b, m, qn):
            ps = pps.tile([128, 512], F32, tag="ps", name="ps_qk")
            for kk in range(8):
                nc.tensor.matmul(
                    ps,
                    wqk_sb[:, kk * 2 * FPC + m * 128:
                           kk * 2 * FPC + (m + 1) * 128],
                    xt_sb[b][qn][:, kk * 512:(kk + 1) * 512],
                    start=(kk == 0), stop=(kk == 7))
            nc.vector.tensor_scalar_add(
                qk_sb[b][m][qn], ps, bqk_sb[:, m:m + 1])

        def emit_v(b, sn):
            ps = pps.tile([128, FPC], F32, tag="ps", name="ps_v")
            cq, off = sn // 4, (sn % 4) * 128
            for kk in range(9):
                lhsT = (xt_sb[b][cq][:, kk * 512 + off:kk * 512 + off + 128]
                        if kk < 8 else ones_full[:, 0:128])
                nc.tensor.matmul(
                    ps, lhsT, wv_sb[:, kk * FPC:(kk + 1) * FPC],
                    start=(kk == 0), stop=(kk == 8))
            vt = v_sb[b * NKB + sn]
            nc.vector.tensor_copy(
                vt.rearrange("p (h w) -> p h w", h=HPC)[:, :, 0:HD],
                ps.rearrange("p (h w) -> p h w", h=HPC))

        def emit_attn(b, h, qh, fillers=()):
            # fillers: [(slot, thunk)]; popped at kb >= slot, so a filler's
            # products may only be consumed at kb >= slot (or later groups)
            fillers = [e if isinstance(e, tuple) else (0, e)
                       for e in fillers]
            pb = h * 64
            ps_o = [pso.tile([VW, 512], F32, tag="pso",
                             name=f"ps_o{q2}") for q2 in range(2)]
            for kb in range(NKB):
                while fillers and fillers[0][0] <= kb:
                    fillers.pop(0)[1]()
                ps_s = pbig.tile([128, 1024], F32, tag="scores",
                                 name="ps_s")
                for q2 in range(2):
                    qc = qh * 2 + q2
                    nc.tensor.matmul(
                        ps_s[:, q2 * 512:(q2 + 1) * 512],
                        qk_sb[b][1][kb // 4][pb:pb + 64,
                                             (kb % 4) * 128:
                                             (kb % 4 + 1) * 128],
                        qk_sb[b][0][qc][pb:pb + 64, :],
                        start=True, stop=True)
                ex = pexp.tile([128, 1024], BF16, tag="expT",
                               name="expT")
                nc.scalar.activation(ex, ps_s, Exp)
                for q2 in range(2):
                    nc.tensor.matmul(
                        ps_o[q2],
                        v_sb[b * NKB + kb][:, h * VW:(h + 1) * VW],
                        ex[:, q2 * 512:(q2 + 1) * 512],
                        start=(kb == 0), stop=(kb == NKB - 1))
            for _, f in fillers:
                f()
            # normalization: recip of sums (psum row 64), fp32r K=1
            # broadcast matmul, copy-out + multiply
            rec_s = pwork.tile([VW, 1024], mybir.dt.float32r,
                               tag="rec", name="rec_s")
            with nc.allow_low_precision(
                    reason="softmax denom recip rounded to f32r "
                           "for the PE broadcast"):
                for q2 in range(2):
                    nc.vector.reciprocal(
                        rec_s[HD:VW, q2 * 512:(q2 + 1) * 512],
                        ps_o[q2][HD:VW, :])
            otmps = []
            for q2 in range(2):
                otmp = pwork.tile([HD, 512], F32, tag="otmp",
                                  name="otmp")
                nc.vector.tensor_copy(otmp, ps_o[q2][0:HD, :])
                otmps.append(otmp)
            for q2 in range(2):
                qc = qh * 2 + q2
                bc_ps = pso.tile([HD, 512], F32, tag="pso",
                                 name="bc_ps")
                nc.tensor.matmul(
                    bc_ps,
                    ones32[HD:VW, :],
                    rec_s[HD:VW, q2 * 512:(q2 + 1) * 512],
                    start=True, stop=True)
                nc.vector.tensor_mul(
                    att_sb[b][h][:, qc * 512:(qc + 1) * 512],
                    otmps[q2], bc_ps[:, :])

        def emit_ship(b, h):
            nc.sync.dma_start(
                out=a2a_in[h][b * 4:(b + 1) * 4, :, :].rearrange(
                    "j p s -> p j s"),
                in_=att_sb[b][h].rearrange("p (j s) -> p j s", j=4))

        def F(fn, *a):
            return lambda: fn(*a)

        # Filler safety rule: a filler popped at kb-slot i is emitted just
        # before slot i's scores, so anything it produces may only be
        # consumed at kb >= i (or by a later group).
        emit_qk(0, 1, 0)
        emit_qk(0, 1, 1)
        emit_qk(0, 0, 0)
        emit_qk(0, 0, 1)
        emit_attn(0, 0, 0, fillers=(
            [(sn, F(emit_v, 0, sn)) for sn in range(8)]
            + [(8, F(emit_qk, 0, 1, 2))]
            + [(sn, F(emit_v, 0, sn)) for sn in range(8, 12)]
            + [(12, F(emit_qk, 0, 1, 3))]
            + [(sn, F(emit_v, 0, sn)) for sn in range(12, NKB)]))
        emit_qk(0, 0, 2)
        emit_qk(0, 0, 3)
        emit_attn(0, 1, 0)
        emit_attn(0, 0, 1, fillers=(
            [(i * 2, F(emit_qk, 1, 1, qn))
             for i, qn in enumerate(range(NQC))]
            + [(8, F(emit_qk, 1, 0, 0)), (10, F(emit_qk, 1, 0, 1))]))
        emit_ship(0, 0)
        emit_attn(0, 1, 1, fillers=(
            [(sn * 2, F(emit_v, 1, sn)) for sn in range(8)]
            + [(14, F(emit_qk, 1, 0, 2)), (15, F(emit_qk, 1, 0, 3))]))
        emit_ship(0, 1)
        emit_attn(1, 0, 0, fillers=[(sn - 8, F(emit_v, 1, sn))
                                    for sn in range(8, NKB)])
        emit_attn(1, 0, 1)
        emit_ship(1, 0)
        emit_a2a(0)
        srcb = a2a_out if with_collective else a2a_in
        for j in range(4):
            nc.sync.dma_start(out=ao_sb[j][0:HD, :],
                              in_=srcb[0][2 * j, :, :])
            nc.sync.dma_start(out=ao_sb[j][HD:128, :],
                              in_=srcb[0][2 * j + 1, :, :])
        emit_attn(1, 1, 0)

        # first half of the output projection (even-head features),
        # spread through the final attention group as fillers
        def emit_out1(g):
            sm, en = g // 2, g % 2
            ps = pps.tile([128, 512], F32, tag="ps", name="ps_out1")
            for kk in range(4):
                nc.tensor.matmul(
                    ps, ao_sb[kk][:, sm * 128:(sm + 1) * 128],
                    wout_sb[kk][:, en * 512:(en + 1) * 512],
                    start=(kk == 0), stop=(kk == 3))
            nc.vector.tensor_copy(part_sb[g], ps)

        emit_attn(1, 1, 1, fillers=[(2 * g, F(emit_out1, g))
                                    for g in range(8)])
        emit_ship(1, 1)
        emit_a2a(1)

        # ---------------- output projection, second half ----------------
        for j in range(4):
            nc.sync.dma_start(out=ao_sb[4 + j][0:HD, :],
                              in_=srcb[1][2 * j, :, :])
            nc.sync.dma_start(out=ao_sb[4 + j][HD:128, :],
                              in_=srcb[1][2 * j + 1, :, :])
        for g in range(8):
            sm, en = g // 2, g % 2
            ps = pps.tile([128, 512], F32, tag="ps", name="ps_out2")
            for kk in range(4, 9):
                lhsT = (ao_sb[kk][:, sm * 128:(sm + 1) * 128] if kk < 8
                        else ones_full[:, sm * 128:(sm + 1) * 128])
                nc.tensor.matmul(
                    ps, lhsT, wout_sb[kk][:, en * 512:(en + 1) * 512],
                    start=(kk == 4), stop=(kk == 8))
            osb = pwork.tile([128, 512], F32, tag="outsb", name="osb")
            nc.vector.tensor_add(osb, ps, part_sb[g])
            nc.sync.dma_start(
                out=out[sm * 128:(sm + 1) * 128, en * 512:(en + 1) * 512],
                in_=osb)

    nc.compile()
    return nc


_NC_CACHE = {}


def _get_nc(with_collective: bool = True):
    key = bool(with_collective)
    if key not in _NC_CACHE:
        _NC_CACHE[key] = _build_nc(with_collective)
    return _NC_CACHE[key]


def make_in_maps(x, w_qkv, b_qkv, w_out, b_out):
    """Host-side sharding/prep. Returns per-core input dicts."""
    x = np.asarray(x, dtype=np.float32)
    w_qkv = np.asarray(w_qkv, dtype=np.float32)
    b_qkv = np.asarray(b_qkv, dtype=np.float32)
    w_out = np.asarray(w_out, dtype=np.float32)
    b_out = np.asarray(b_out, dtype=np.float32)

    wq = w_qkv[0:D].reshape(H, HD, D)
    wk = w_qkv[D:2 * D].reshape(H, HD, D)
    wv = w_qkv[2 * D:3 * D].reshape(H, HD, D)
    bq = b_qkv[0:D].reshape(H, HD)
    bk = b_qkv[D:2 * D].reshape(H, HD)
    bv = b_qkv[2 * D:3 * D].reshape(H, HD)
    scale = 1.0 / np.sqrt(HD)

    wout_t = np.empty((D + 128, D), dtype=NPBF16)
    perm = np.concatenate(
        [np.arange(h * HD, (h + 1) * HD) for h in range(0, H, 2)]
        + [np.arange(h * HD, (h + 1) * HD) for h in range(1, H, 2)])
    wout_t[0:D] = w_out.T[perm].astype(NPBF16)
    wout_t[D:] = (b_out / 128.0).astype(NPBF16)[None, :]

    # [d, 4096] stacked batch-major
    xt_all = np.ascontiguousarray(
        np.concatenate([x[0].T, x[1].T], axis=1)).astype(NPBF16)

    in_maps = []
    for c in range(NCORES):
        hs = slice(c * HPC, (c + 1) * HPC)
        wq_c = (wq[hs].reshape(FPC, D) * scale).T
        wk_c = wk[hs].reshape(FPC, D).T
        wqk_c = np.concatenate([wq_c, wk_c], axis=1).astype(NPBF16)
        bqk_c = np.concatenate([bq[hs].reshape(FPC) * scale,
                                bk[hs].reshape(FPC)])
        bqk_c = np.ascontiguousarray(
            bqk_c.reshape(2, 128).T).astype(np.float32)
        wv_c = np.empty((D + 128, FPC), dtype=NPBF16)
        wv_c[0:D] = wv[hs].reshape(FPC, D).T.astype(NPBF16)
        wv_c[D:] = (bv[hs].reshape(FPC) / 128.0).astype(NPBF16)[None, :]
        in_maps.append({
            "onesr": np.ones((VW, HD), dtype=np.float32),
            "xt": xt_all,
            "wqk": np.ascontiguousarray(wqk_c),
            "bqk": bqk_c,
            "wv": np.ascontiguousarray(wv_c),
            "wout": wout_t,
        })
    return in_maps


def assemble_output(results):
    out = np.empty((B, S, D), dtype=np.float32)
    for c in range(NCORES):
        b, sg = c // 4, c % 4
        out[b, sg * SC:(sg + 1) * SC, :] = results[c]["out"]
    return out


def kernel(x, mask, w_qkv, b_qkv, w_out, b_out):
    nc = _get_nc(True)
    in_maps = make_in_maps(x, w_qkv, b_qkv, w_out, b_out)
    res = run_bass_kernel_spmd(nc, in_maps, core_ids=list(range(NCORES)))
    return assemble_output(res.results)
